# revision 18
# baseline (speedup 1.0000x reference)
"""CMoEGenerator Trainium2 kernel.

Reference computation (B=32, K=8, S=512, HS=256):
    rem_lin = rem_fea @ Wr + br                  # [B,S,D]
    ret_lin = ret_fea @ Wt + bt                  # [B,K,S,D]
    scores[b,k] = mean_s(rem_lin)[b] . mean_s(ret_lin)[b,k]
    routing = softmax_k(scores)
    h = relu(ret_fea @ W1[k] + b1[k])
    expert = h @ W2[k] + b2[k]
    gen[b] = sum_k routing[b,k] * expert[b,k]

Key algebraic simplification: mean_s commutes with the linear layers, so
    mean_s(rem_lin)[b]   = (mean_s rem_fea[b]) @ Wr + br
    mean_s(ret_lin)[b,k] = (mean_s ret_fea[b,k]) @ Wt + bt
which removes the two large routing matmuls entirely.

Sharding: data-parallel over B across 8 cores (4 batches/core, weights
replicated, no collectives).

Per-core dataflow (P=128 partitions, SC=4 s-chunks, DC=2 d-chunks):
  - ret_fea is shipped bf16 and transposed during the HBM->SBUF DMA via the
    xbar (DMA-transpose): per batch, 2 big DMAs, one per 128-wide d-block,
    [K*S, 128] -> XT [128, K*S]. No PE transposes, no PSUM evictions.
  - Routing input sums v[b,k,:] = sum_s X: one DVE free-axis reduce per
    (batch, d-block) over the [128, K*S] XT tile.
  - MM1: Y.T[e,s] = W1[k].T @ X.T  (lhsT = W1 natural chunks, rhs = XT view)
  - relu eviction on the ACT engine applies scale=routing[b,k] (>0, commutes
    with relu) and bias=routing[b,k]*b1[k] in a single pass -> hT (bf16).
  - MM2: gen[s,d] accumulates over all (k, e-chunk) in PSUM
    (lhsT = hT chunks, rhs = W2 natural), plus a final rank-8 matmul adding
    ones(s) x (sum_k routing[b,k] b2[k]) for the expert biases.
  - One [P, S*D/P] eviction + one DMA out per batch.
  - All heavy matmuls run bf16 (full PE rate); routing math in fp32.
"""

import numpy as np

B, K, S, D = 32, 8, 512, 256
NCORES = 8
BC = B // NCORES  # batches per core
P = 128
SC = S // P  # 4 s-chunks
DC = D // P  # 2 d-chunks

_CACHE = {}


def _build():
    import concourse.bacc as bacc
    import concourse.mybir as mybir
    import concourse.tile as tile

    f32 = mybir.dt.float32
    bf16 = mybir.dt.bfloat16
    AF = mybir.ActivationFunctionType
    ALU = mybir.AluOpType

    nc = bacc.Bacc("TRN2", target_bir_lowering=False, debug=False)

    ret_t = nc.dram_tensor("ret_fea", [BC, K, S, D], bf16, kind="ExternalInput")
    rem_t = nc.dram_tensor("rem_fea", [BC, S, D], bf16, kind="ExternalInput")
    Wr_t = nc.dram_tensor("Wr", [D, D], f32, kind="ExternalInput")
    br_t = nc.dram_tensor("br", [D], f32, kind="ExternalInput")
    Wt_t = nc.dram_tensor("Wt", [D, D], f32, kind="ExternalInput")
    bt_t = nc.dram_tensor("bt", [D], f32, kind="ExternalInput")
    W1_t = nc.dram_tensor("W1", [K, D, D], bf16, kind="ExternalInput")
    b1T_t = nc.dram_tensor("b1T", [D, K], f32, kind="ExternalInput")
    W2_t = nc.dram_tensor("W2", [K, D, D], bf16, kind="ExternalInput")
    b2_t = nc.dram_tensor("b2", [K, D], bf16, kind="ExternalInput")
    out_t = nc.dram_tensor("gen_fea", [BC, S, D], f32, kind="ExternalOutput")

    ret = ret_t.ap()
    rem = rem_t.ap()
    out = out_t.ap()

    with tile.TileContext(nc) as tc:
        with (
            tc.tile_pool(name="consts", bufs=1) as consts,
            tc.tile_pool(name="xt", bufs=2 * K * DC + 2) as xtpool,
            tc.tile_pool(name="rempool", bufs=2) as rempool,
            tc.tile_pool(name="ht", bufs=4) as htpool,
            tc.tile_pool(name="gen", bufs=2) as genpool,
            tc.tile_pool(name="small", bufs=2) as small,
            tc.tile_pool(name="yp", bufs=3, space="PSUM") as yp,
            tc.tile_pool(name="genp", bufs=2, space="PSUM") as genp,
            tc.tile_pool(name="tinyp", bufs=1, space="PSUM") as tinyp,
        ):
            # ---- one-time constants ----
            ones_col = consts.tile([P, 1], bf16, tag="ones_col")  # value 1/S
            nc.gpsimd.memset(ones_col, 0.0)
            nc.gpsimd.affine_select(
                out=ones_col,
                in_=ones_col,
                compare_op=ALU.not_equal,
                fill=1.0 / S,
                base=0,
                pattern=[[0, 1]],
                channel_multiplier=0,
            )
            ones_row = consts.tile([1, P], f32, tag="ones_row")  # value 1.0
            nc.vector.memset(ones_row, 1.0)

            W1_sb = consts.tile([P, K, DC, D], bf16, tag="w1")
            W2_sb = consts.tile([P, K, DC, D], bf16, tag="w2")
            W1_view = W1_t.ap().rearrange("k (dc p) e -> p k dc e", p=P)
            W2_view = W2_t.ap().rearrange("k (dc p) e -> p k dc e", p=P)

            Wr_sb = consts.tile([P, DC, D], f32, tag="wr")
            nc.sync.dma_start(
                out=Wr_sb, in_=Wr_t.ap().rearrange("(dc p) e -> p dc e", p=P)
            )
            # Wt is used only for routing; fold the 1/S mean normalization of
            # the expert input sums into it after load.
            Wt_sb = consts.tile([P, DC, D], f32, tag="wt")
            nc.sync.dma_start(
                out=Wt_sb, in_=Wt_t.ap().rearrange("(dc p) e -> p dc e", p=P)
            )
            nc.vector.tensor_scalar_mul(Wt_sb, Wt_sb, 1.0 / S)

            b1T_sb = consts.tile([P, DC, K], f32, tag="b1T")
            nc.sync.dma_start(
                out=b1T_sb, in_=b1T_t.ap().rearrange("(dc p) k -> p dc k", p=P)
            )
            b2_sb = consts.tile([K, D], bf16, tag="b2")
            nc.sync.dma_start(out=b2_sb, in_=b2_t.ap())
            br_sb = consts.tile([1, D], f32, tag="br")
            nc.sync.dma_start(out=br_sb, in_=br_t.ap()[None, :])
            bt_sb = consts.tile([1, D], f32, tag="bt")
            nc.sync.dma_start(out=bt_sb, in_=bt_t.ap()[None, :])

            # ---- software-pipelined per-batch schedule ----
            def stage_load(b):
                """DMA one batch's inputs; X arrives transposed via the xbar.
                Per-(k,dc) tiles so downstream reduces/matmuls unblock as each
                expert's slab lands rather than waiting for the whole batch."""
                rem_sb = rempool.tile([P, SC, D], bf16, tag="rem")
                nc.sync.dma_start(
                    out=rem_sb,
                    in_=rem[b].rearrange("(p sc) d -> p sc d", p=P),
                )
                XT = []
                for k in range(K):
                    XT_dc = []
                    for dc in range(DC):
                        xt = xtpool.tile([P, S], bf16, tag="xt")
                        nc.sync.dma_start(
                            out=xt,
                            in_=ret[b, k][:, dc * P : (dc + 1) * P],
                            transpose=True,
                        )
                        XT_dc.append(xt)
                    XT.append(XT_dc)
                return rem_sb, XT

            def stage_u(st):
                """uT[d,1] = (mean_s rem[b]) transposed, via tiny PE matmuls."""
                rem_sb = st["rem"]
                u_psum = tinyp.tile([1, D], f32, tag="scr")
                for sc in range(SC):
                    nc.tensor.matmul(
                        u_psum,
                        ones_col,
                        rem_sb[:, sc, :],
                        start=(sc == 0),
                        stop=(sc == SC - 1),
                    )
                u_sb = small.tile([1, D], f32, tag="u")
                nc.scalar.copy(u_sb, u_psum)
                uT_psum = tinyp.tile([P, DC], f32, tag="scr")
                for dc in range(DC):
                    nc.tensor.transpose(
                        uT_psum[:, dc : dc + 1],
                        u_sb[:, dc * P : (dc + 1) * P],
                        ones_row[:1, :1],
                    )
                uT_sb = small.tile([P, DC], f32, tag="uT")
                nc.vector.tensor_copy(uT_sb, uT_psum)
                st["uT"] = uT_sb

            def stage_v_k(st, k):
                """Expert k's input sums v.T[d] via DVE free-axis reduces."""
                if "vTa" not in st:
                    st["vTa"] = small.tile(
                        [P, DC, K], f32, tag="vTa", name="vTa_sb"
                    )
                for dc in range(DC):
                    nc.vector.reduce_sum(
                        st["vTa"][:, dc, k : k + 1],
                        st["XT"][k][dc],
                        axis=mybir.AxisListType.X,
                    )

            def stage_routing(st):
                uT_sb = st["uT"]
                vT_sb = st["vTa"]
                art_psum = tinyp.tile([P, DC, K], f32, tag="scr")
                for ec in range(DC):
                    for dc in range(DC):
                        nc.tensor.matmul(
                            art_psum[:, ec, :],
                            Wt_sb[:, dc, ec * P : (ec + 1) * P],
                            vT_sb[:, dc, :],
                            start=(dc == 0),
                            stop=False,
                        )
                    nc.tensor.matmul(
                        art_psum[:, ec, :],
                        bt_sb[:, ec * P : (ec + 1) * P],
                        ones_row[:, :K],
                        start=False,
                        stop=True,
                    )
                art_sb = small.tile([P, DC, K], f32, tag="art")
                nc.vector.tensor_copy(art_sb, art_psum)

                arm_psum = tinyp.tile([P, DC], f32, tag="scr")
                for ec in range(DC):
                    for dc in range(DC):
                        nc.tensor.matmul(
                            arm_psum[:, ec : ec + 1],
                            Wr_sb[:, dc, ec * P : (ec + 1) * P],
                            uT_sb[:, dc : dc + 1],
                            start=(dc == 0),
                            stop=False,
                        )
                    nc.tensor.matmul(
                        arm_psum[:, ec : ec + 1],
                        br_sb[:, ec * P : (ec + 1) * P],
                        ones_row[:, :1],
                        start=False,
                        stop=True,
                    )
                arm_sb = small.tile([P, DC], f32, tag="arm")
                nc.vector.tensor_copy(arm_sb, arm_psum)

                sc_psum = tinyp.tile([1, K], f32, tag="scr")
                for ec in range(DC):
                    nc.tensor.matmul(
                        sc_psum,
                        arm_sb[:, ec : ec + 1],
                        art_sb[:, ec, :],
                        start=(ec == 0),
                        stop=(ec == DC - 1),
                    )
                sc_sb = small.tile([1, K], f32, tag="scores")
                nc.scalar.copy(sc_sb, sc_psum)

                # softmax over k (scores are O(1); skip max subtraction)
                exps = small.tile([1, K], f32, tag="exps")
                nc.scalar.activation(exps, sc_sb, AF.Exp)
                ssum = small.tile([1, 1], f32, tag="ssum")
                nc.vector.reduce_sum(ssum, exps, axis=mybir.AxisListType.X)
                sinv = small.tile([1, 1], f32, tag="sinv")
                nc.vector.reciprocal(sinv, ssum)
                routing = small.tile([1, K], f32, tag="routing")
                nc.vector.tensor_scalar_mul(routing, exps, sinv)

                rbc_psum = tinyp.tile([P, K], f32, tag="scr")
                nc.tensor.matmul(rbc_psum, ones_row, routing, start=True, stop=True)
                r_all = small.tile([P, K], f32, tag="r_all")
                nc.vector.tensor_copy(r_all, rbc_psum)

                rtb_psum = tinyp.tile([K, P], f32, tag="scr")
                nc.tensor.matmul(rtb_psum, routing, ones_row, start=True, stop=True)
                rtb_sb = small.tile([K, P], bf16, tag="rtb")
                nc.vector.tensor_copy(rtb_sb, rtb_psum)

                rb1_sb = small.tile([P, DC, K], f32, tag="rb1")
                for k in range(K):
                    nc.vector.tensor_scalar_mul(
                        rb1_sb[:, :, k], b1T_sb[:, :, k], r_all[:, k : k + 1]
                    )
                st["r_all"], st["rtb"], st["rb1"] = r_all, rtb_sb, rb1_sb

            def compute_k(st, k):
                """MM1 -> scaled relu -> MM2 partial accumulation for expert k."""
                if st["genp"] is None:
                    st["genp"] = genp.tile([P, SC, D], f32, tag="gps", name="g_ps")
                g_ps = st["genp"]
                ys = []
                for ec in range(DC):
                    y_ps = yp.tile([P, S], f32, tag="yps")
                    for dc in range(DC):
                        nc.tensor.matmul(
                            y_ps,
                            W1_sb[:, k, dc, ec * P : (ec + 1) * P],
                            st["XT"][k][dc],
                            start=(dc == 0),
                            stop=(dc == DC - 1),
                        )
                    ys.append(y_ps)
                hT = htpool.tile([P, DC, S], bf16, tag="ht")
                for ec in range(DC):
                    # hT = relu(routing[k] * (Y + b1[k]))
                    nc.scalar.activation(
                        out=hT[:, ec, :],
                        in_=ys[ec],
                        func=AF.Relu,
                        bias=st["rb1"][:, ec, k : k + 1],
                        scale=st["r_all"][:, k : k + 1],
                    )
                for ec in range(DC):
                    for sc in range(SC):
                        nc.tensor.matmul(
                            g_ps[:, sc, :],
                            hT[:, ec, sc * P : (sc + 1) * P],
                            W2_sb[:, k, ec, :],
                            start=(k == 0 and ec == 0 and sc % 2 == 0),
                            stop=False,
                        )

            def finish_b(st, b):
                g_ps = st["genp"]
                for sc in range(SC):
                    nc.tensor.matmul(
                        g_ps[:, sc, :],
                        st["rtb"],
                        b2_sb,
                        start=False,
                        stop=(sc % 2 == 1),
                    )
                gen_sb = genpool.tile([P, SC, D], f32, tag="gen")
                nc.vector.tensor_copy(gen_sb, g_ps)
                # XT's s-chunks are contiguous (s = sc*P + p), unlike the
                # "(p sc)" interleave a PE-transpose pipeline would produce.
                nc.sync.dma_start(
                    out=out[b].rearrange("(sc p) d -> p sc d", p=P),
                    in_=gen_sb,
                )

            def new_state(b, rem_sb, XT):
                return {"b": b, "rem": rem_sb, "XT": XT, "genp": None}

            # prologue: stage batch 0; steady loop interleaves next-batch
            # staging with compute and emits routing(b+1) late in the k-loop
            # so its PE ops never stall the PE queue.
            rem_sb, XT = stage_load(0)
            nc.sync.dma_start(out=W1_sb, in_=W1_view)
            nc.sync.dma_start(out=W2_sb, in_=W2_view)
            cur = new_state(0, rem_sb, XT)
            stage_u(cur)
            for k in range(K):
                stage_v_k(cur, k)
            stage_routing(cur)

            for b in range(BC):
                nxt = None
                if b + 1 < BC:
                    rem_sb, XT = stage_load(b + 1)
                    nxt = new_state(b + 1, rem_sb, XT)
                for k in range(K):
                    compute_k(cur, k)
                    if nxt is not None:
                        if k < 4:
                            stage_v_k(nxt, 2 * k)
                            stage_v_k(nxt, 2 * k + 1)
                        if k == 1:
                            stage_u(nxt)
                        elif k == 3:
                            stage_routing(nxt)
                finish_b(cur, b)
                cur = nxt

    nc.compile()
    return nc


def _make_in_maps(rem_fea, ret_fea, Wr, br, Wt, bt, W1, b1, W2, b2):
    import ml_dtypes

    bf16 = ml_dtypes.bfloat16
    rem_fea = np.ascontiguousarray(np.asarray(rem_fea, dtype=np.float32).astype(bf16))
    ret_fea = np.ascontiguousarray(np.asarray(ret_fea, dtype=np.float32).astype(bf16))
    shared = {
        "Wr": np.ascontiguousarray(np.asarray(Wr, np.float32)),
        "br": np.ascontiguousarray(np.asarray(br, np.float32)),
        "Wt": np.ascontiguousarray(np.asarray(Wt, np.float32)),
        "bt": np.ascontiguousarray(np.asarray(bt, np.float32)),
        "W1": np.ascontiguousarray(np.asarray(W1, np.float32).astype(bf16)),
        "b1T": np.ascontiguousarray(np.asarray(b1, np.float32).T),
        "W2": np.ascontiguousarray(np.asarray(W2, np.float32).astype(bf16)),
        "b2": np.ascontiguousarray(np.asarray(b2, np.float32).astype(bf16)),
    }
    in_maps = []
    for c in range(NCORES):
        sl = slice(c * BC, (c + 1) * BC)
        in_maps.append(
            {
                "rem_fea": rem_fea[sl],
                "ret_fea": ret_fea[sl],
                **shared,
            }
        )
    return in_maps


def run(in_maps, **kwargs):
    from concourse.bass_utils import run_bass_kernel_spmd

    if "nc" not in _CACHE:
        _CACHE["nc"] = _build()
    return run_bass_kernel_spmd(
        _CACHE["nc"], in_maps, core_ids=list(range(NCORES)), **kwargs
    )


def _get_runner():
    """Build (once) a cached jitted SPMD executable over 8 cores.

    Mirrors concourse.bass2jax.run_bass_via_pjrt's multi-core path, but keeps
    the jitted function cached so repeated kernel() calls don't re-trace.
    Outputs are NOT donated: the kernel writes every element of gen_fea, so
    the zero output operands can be staged once and reused across calls.
    """
    if "runner" in _CACHE:
        return _CACHE["runner"]

    import jax
    import jax.numpy as jnp  # noqa: F401
    from jax.sharding import Mesh, NamedSharding, PartitionSpec

    try:
        from jax import shard_map
    except ImportError:
        from jax.experimental.shard_map import shard_map

    import concourse.mybir as mybir
    from concourse import bass2jax

    bass2jax.install_neuronx_cc_hook()
    if "nc" not in _CACHE:
        _CACHE["nc"] = _build()
    nc = _CACHE["nc"]

    partition_name = (
        nc.partition_id_tensor.name if nc.partition_id_tensor else None
    )
    in_names = []
    out_names = []
    out_avals = []
    zero_shapes = []
    for alloc in nc.m.functions[0].allocations:
        if not isinstance(alloc, mybir.MemoryLocationSet):
            continue
        name = alloc.memorylocations[0].name
        if alloc.kind == "ExternalInput":
            if name != partition_name:
                in_names.append(name)
        elif alloc.kind == "ExternalOutput":
            out_names.append(name)
            shape = tuple(alloc.tensor_shape)
            dtype = mybir.dt.np(alloc.dtype)
            out_avals.append(jax.core.ShapedArray(shape, dtype))
            zero_shapes.append((shape, dtype))
    n_params = len(in_names)
    all_names = in_names + out_names
    if partition_name is not None:
        all_names.append(partition_name)

    def _body(*args):
        operands = list(args)
        if partition_name is not None:
            operands.append(bass2jax.partition_id_tensor())
        outs = bass2jax._bass_exec_p.bind(
            *operands,
            out_avals=tuple(out_avals),
            in_names=tuple(all_names),
            out_names=tuple(out_names),
            lowering_input_output_aliases=(),
            sim_require_finite=True,
            sim_require_nnan=True,
            nc=nc,
        )
        return tuple(outs)

    devices = jax.devices()[:NCORES]
    mesh = Mesh(np.asarray(devices), ("core",))
    n_ops = n_params + len(out_names)
    specs = (PartitionSpec("core"),) * n_ops
    out_specs = (PartitionSpec("core"),) * len(out_names)
    try:
        smfn = shard_map(
            _body, mesh=mesh, in_specs=specs, out_specs=out_specs,
            check_vma=False,
        )
    except TypeError:
        smfn = shard_map(
            _body, mesh=mesh, in_specs=specs, out_specs=out_specs,
            check_rep=False,
        )

    sharding = NamedSharding(mesh, PartitionSpec("core"))
    zeros_dev = [
        jax.device_put(np.zeros((NCORES * s[0], *s[1:]), d), sharding)
        for s, d in zero_shapes
    ]
    for z in zeros_dev:
        z.block_until_ready()

    abstract_in = []
    for alloc in nc.m.functions[0].allocations:
        if not isinstance(alloc, mybir.MemoryLocationSet):
            continue
        name = alloc.memorylocations[0].name
        if alloc.kind == "ExternalInput" and name != partition_name:
            shape = tuple(alloc.tensor_shape)
            abstract_in.append(
                jax.ShapeDtypeStruct(
                    (NCORES * shape[0], *shape[1:]), mybir.dt.np(alloc.dtype),
                    sharding=sharding,
                )
            )

    def compile_fn():
        return (
            jax.jit(smfn)
            .lower(*abstract_in, *[jax.ShapeDtypeStruct(z.shape, z.dtype, sharding=sharding) for z in zeros_dev])
            .compile()
        )

    try:
        sharded = bass2jax.fast_dispatch_compile(compile_fn)
    except Exception:
        sharded = jax.jit(smfn)
    _CACHE["runner"] = (
        sharded,
        in_names,
        out_names,
        out_avals,
        zeros_dev,
        sharding,
    )
    return _CACHE["runner"]


def _run_cached(in_maps):
    sharded, in_names, out_names, out_avals, zeros_dev, _sh = _get_runner()
    concat_in = [
        np.concatenate([np.asarray(in_maps[c][nm]) for c in range(NCORES)], axis=0)
        for nm in in_names
    ]
    out_arrs = sharded(*concat_in, *zeros_dev)
    return {
        nm: np.asarray(out_arrs[i]).reshape(NCORES, *out_avals[i].shape)
        for i, nm in enumerate(out_names)
    }


def kernel(rem_fea, ret_fea, Wr, br, Wt, bt, W1, b1, W2, b2):
    in_maps = _make_in_maps(rem_fea, ret_fea, Wr, br, Wt, bt, W1, b1, W2, b2)
    try:
        outs = _run_cached(in_maps)
        return np.concatenate(list(outs["gen_fea"]), axis=0)
    except Exception:
        res = run(in_maps)
        return np.concatenate(
            [res.results[c]["gen_fea"] for c in range(NCORES)], axis=0
        )
